# revision 25
# baseline (speedup 1.0000x reference)
"""Fused int8-dequant attention block on 8 Trainium2 NeuronCores.

Reference computation (per nn_Attention_28183575396503):
    q = rope(rms_norm(x @ wq.T * wq_s, qn_w));  k likewise;  v = x @ wv.T * wv_s
    out = softmax(q k^T / sqrt(Dh) + len_mask) v  @ wo.T * wo_s

Sharding: tensor-parallel over heads (4-way, 4 heads / 512 channels per core)
x data-parallel over batch (2-way).  Core c -> (b = c//4, head-group = c%4).
The RMS-norm sum-of-squares couples all 2048 channels, so each 4-core TP group
does one tiny AllReduce of the per-row partial sums ([1, 2S] fp32).  The
row-parallel out-projection partials are summed on the host (the standard
Megatron all-reduce, done during unshard).

All matmuls run in bf16 (int8 weights are exact in bf16) with fp32 PSUM
accumulation; softmax runs unnormalized-exp (|scores| <~ 7 after RMS norm)
with the length mask folded into the exp bias (per-partition bias on the ACT
op).  The elementwise rope/norm path runs in bf16 for the DVE 2x rate; wv/wo
dequant scales are folded into the bf16 weights on the host.
"""

import numpy as np
import ml_dtypes

import concourse.bass as bass
import concourse.mybir as mybir
import concourse.tile as tile
from concourse.bass_utils import run_bass_kernel_spmd

# ---------------------------------------------------------------------------
# Problem constants (hardcoded per the task contract)
B, S, D, H, Dh = 2, 1024, 2048, 16, 128
EPS = 1e-6
NCORES, TP = 8, 4
DL = D // TP            # 512 local channels per core
HL = H // TP            # 4 local heads per core
NDT = D // 128          # 16 contraction tiles
NOT_ = DL // 128        # 4 local out tiles for q/k
NST = S // 128          # 8 sequence tiles
NSC = S // 512          # 2 sequence chunks of 512
BF16 = mybir.dt.bfloat16
F32 = mybir.dt.float32
MASK_NEG = -30000.0

LAST_RESULTS = None     # BassKernelResults of the most recent kernel() call


# ---------------------------------------------------------------------------
# Workaround: this walrus build accepts only ONE sync-wait per instruction,
# but Tile freely emits several (and puts every engine's tail wait on the
# kernel-end Drain).  Post-pass: spill excess waits onto same-engine NoOps
# inserted immediately before the instruction (engines run in program order,
# so waiting on the NoOps first is equivalent).
_MAX_WAITS = 1


def _split_excess_waits(nc: bass.Bass) -> None:
    ctr = 0
    for f in nc.m.functions:
        for bb in f.blocks:
            il = bb.instructions
            new = []
            for inst in il:
                si = inst.sync_info
                ow = list(si.on_wait) if si is not None and si.on_wait else []
                if len(ow) > _MAX_WAITS:
                    for w in ow[_MAX_WAITS:]:
                        ctr += 1
                        new.append(
                            mybir.InstNoOp(
                                name=f"{inst.name}_sw{ctr}",
                                sync_info=mybir.SyncInfo(on_wait=[w], on_update=[]),
                                bass_nofuse=True,
                                engine=inst.engine,
                            )
                        )
                    inst.sync_info = mybir.SyncInfo(
                        on_wait=ow[:_MAX_WAITS],
                        on_update=list(si.on_update) if si.on_update else [],
                    )
                new.append(inst)
            il[:] = new


# ---------------------------------------------------------------------------
def build_nc(apply_qn: bool) -> bass.Bass:
    from contextlib import ExitStack

    AF = mybir.ActivationFunctionType
    OP = mybir.AluOpType

    nc = bass.Bass()
    xT = nc.dram_tensor("xT", [D, S], BF16, kind="ExternalInput")
    wqT = nc.dram_tensor("wqT", [D, DL], BF16, kind="ExternalInput")
    wkT = nc.dram_tensor("wkT", [D, DL], BF16, kind="ExternalInput")
    wvT = nc.dram_tensor("wvT", [D, DL], BF16, kind="ExternalInput")
    woT = nc.dram_tensor("woT", [DL, D], BF16, kind="ExternalInput")
    qs = nc.dram_tensor("qs", [128, NOT_], F32, kind="ExternalInput")
    ks = nc.dram_tensor("ks", [128, NOT_], F32, kind="ExternalInput")
    cosq = nc.dram_tensor("cosq", [128, S], BF16, kind="ExternalInput")
    sinq = nc.dram_tensor("sinq", [128, S], BF16, kind="ExternalInput")
    cosk = nc.dram_tensor("cosk", [128, S], BF16, kind="ExternalInput")
    sink = nc.dram_tensor("sink", [128, S], BF16, kind="ExternalInput")
    maskb = nc.dram_tensor("maskb", [128, NST], F32, kind="ExternalInput")
    if apply_qn:
        qkn = nc.dram_tensor("qkn", [128, 2 * HL], BF16, kind="ExternalInput")
    outp = nc.dram_tensor("outp", [S, D], F32, kind="ExternalOutput")

    with tile.TileContext(nc) as tc, ExitStack() as ctx:
        cpool = ctx.enter_context(tc.tile_pool(name="const", bufs=1))
        dram = ctx.enter_context(tc.tile_pool(name="dram", bufs=1, space="DRAM"))

        # Long-lived SBUF tensors
        q_rope = cpool.tile([128, HL, S], BF16)      # roped/normed q^T per head
        k_rope = cpool.tile([128, HL, S], BF16)
        v_sb = cpool.tile([128, NST, DL], BF16)      # v in [s, o] layout
        attnT = cpool.tile([128, HL, S], BF16)       # attn^T per head
        qs_t = cpool.tile([128, NOT_], F32)
        ks_t = cpool.tile([128, NOT_], F32)
        mask_t = cpool.tile([128, NST], F32)
        ones1 = cpool.tile([128, 1], F32)
        ones128 = cpool.tile([128, 128], BF16)
        eps_t = cpool.tile([128, 1], F32)
        if apply_qn:
            qkn_t = cpool.tile([128, 2 * HL], BF16)

        # Phase-2/3 SBUF pools hoisted above phase 1 so their tiles never
        # overlap phase-1 addresses (stack allocator WAR deps would chain
        # attention starts behind the whole rope phase otherwise).
        ep = ctx.enter_context(tc.tile_pool(name="expp", bufs=3))
        rcp = ctx.enter_context(tc.tile_pool(name="recipp", bufs=1))
        wop = ctx.enter_context(tc.tile_pool(name="wop", bufs=1))
        ostg = ctx.enter_context(tc.tile_pool(name="ostg", bufs=2))
        wo_t = wop.tile([128, HL, D], BF16)

        nc.vector.memset(ones1[:], 1.0)
        nc.vector.memset(ones128[:], 1.0)
        nc.vector.memset(eps_t[:], EPS)
        nc.sync.dma_start(qs_t[:], qs[:, :])
        nc.sync.dma_start(ks_t[:], ks[:, :])
        nc.sync.dma_start(mask_t[:], maskb[:, :])
        if apply_qn:
            nc.sync.dma_start(qkn_t[:], qkn[:, :])

        arin_q = dram.tile([1, S], F32)
        arout_q = dram.tile([1, S], F32)
        arin_k = dram.tile([1, S], F32)
        arout_k = dram.tile([1, S], F32)

        # ------------------------------------------------------------------
        # Phase 1: QKV projections, distributed RMS-norm stats, RoPE
        with ExitStack() as p1:
            opool = p1.enter_context(tc.tile_pool(name="p1o", bufs=1))
            q_raw = opool.tile([128, NOT_, S], BF16)
            k_raw = opool.tile([128, NOT_, S], BF16)
            ssq_t = opool.tile([128, 2 * S], F32)
            r_bf = opool.tile([128, 2 * S], BF16)
            acc_q = opool.tile([128, S], F32)
            acc_k = opool.tile([128, S], F32)

            # ---- p1a: projections + ssq AllReduces + RoPE, FIFO-balanced:
            # DVE: q-epi, k-epi, rope-q, v-copies 0-3, rope-k, v-copies 4-7
            # ACT: ln/exp (r chains) only
            # SP DMA ring: input loads, cos/sin, rot shifts (never blocked)
            # GpSimd: AR staging/trigger/broadcast (the AR-dependent chain)
            with ExitStack() as p1a:
                apool = p1a.enter_context(tc.tile_pool(name="p1a", bufs=1))
                xT_t = apool.tile([128, NDT, S], BF16)
                wq_t = apool.tile([128, NDT, DL], BF16)
                wk_t = apool.tile([128, NDT, DL], BF16)
                wv_t = apool.tile([128, NDT, DL], BF16)
                cq_t = apool.tile([128, S], BF16)
                sq_t = apool.tile([128, S], BF16)
                ck_t = apool.tile([128, S], BF16)
                sk_t = apool.tile([128, S], BF16)

                xTr = xT.rearrange("(t p) s -> p t s", p=128)
                wqr = wqT.rearrange("(t p) o -> p t o", p=128)
                wkr = wkT.rearrange("(t p) o -> p t o", p=128)
                wvr = wvT.rearrange("(t p) o -> p t o", p=128)
                for dt in range(NDT):
                    nc.sync.dma_start(xT_t[:, dt, :], xTr[:, dt, :])
                    nc.sync.dma_start(wq_t[:, dt, :], wqr[:, dt, :])
                    nc.sync.dma_start(wk_t[:, dt, :], wkr[:, dt, :])
                    nc.sync.dma_start(wv_t[:, dt, :], wvr[:, dt, :])
                nc.sync.dma_start(cq_t[:], cosq[:, :])
                nc.sync.dma_start(sq_t[:], sinq[:, :])
                nc.sync.dma_start(ck_t[:], cosk[:, :])
                nc.sync.dma_start(sk_t[:], sink[:, :])

                psum_p = p1a.enter_context(
                    tc.tile_pool(name="psA", bufs=4, space="PSUM")
                )
                psum_s = p1a.enter_context(
                    tc.tile_pool(name="psS", bufs=2, space="PSUM")
                )
                sqpool = p1a.enter_context(tc.tile_pool(name="sqp", bufs=2))
                ropep = p1a.enter_context(tc.tile_pool(name="rope", bufs=2))

                def proj_qk(wt, raw, sc_t, acc, off, ari, aro):
                    for ot in range(NOT_):
                        for sc_ in range(NSC):
                            ps = psum_p.tile([128, 512], F32, tag="proj")
                            for dt in range(NDT):
                                nc.tensor.matmul(
                                    ps[:],
                                    wt[:, dt, ot * 128 : (ot + 1) * 128],
                                    xT_t[:, dt, sc_ * 512 : (sc_ + 1) * 512],
                                    start=(dt == 0),
                                    stop=(dt == NDT - 1),
                                )
                            seg = raw[:, ot, sc_ * 512 : (sc_ + 1) * 512]
                            nc.vector.tensor_scalar_mul(
                                seg, ps[:], sc_t[:, ot : ot + 1]
                            )
                            accseg = acc[:, sc_ * 512 : (sc_ + 1) * 512]
                            if ot == 0:
                                nc.vector.tensor_tensor(
                                    accseg, seg, seg, op=OP.mult
                                )
                            else:
                                sq = sqpool.tile([128, 512], F32, tag="sq")
                                nc.vector.tensor_tensor(sq[:], seg, seg, op=OP.mult)
                                nc.vector.tensor_tensor(
                                    accseg, accseg, sq[:], op=OP.add
                                )
                    # partial sum of squares over the 512 local channels
                    pssq = psum_s.tile([1, S], F32, tag="ssq")
                    for sc_ in range(NSC):
                        nc.tensor.matmul(
                            pssq[0:1, sc_ * 512 : (sc_ + 1) * 512],
                            ones1[:],
                            acc[:, sc_ * 512 : (sc_ + 1) * 512],
                            start=True,
                            stop=True,
                        )
                    # acc row 0 is dead now; reuse it to stage psum -> DRAM.
                    # The whole AR-dependent chain goes through GpSimd queues
                    # so it can never head-of-line-block the SP DMA ring.
                    nc.vector.tensor_copy(out=acc[0:1, :], in_=pssq[0:1, :])
                    nc.gpsimd.dma_start(ari[0:1, :], acc[0:1, :])
                    nc.gpsimd.collective_compute(
                        "AllReduce",
                        OP.add,
                        replica_groups=[[0, 1, 2, 3], [4, 5, 6, 7]],
                        ins=[ari.opt()],
                        outs=[aro.opt()],
                    )
                    nc.gpsimd.dma_start(
                        ssq_t[:, off : off + S],
                        aro[0:1, :].broadcast_to((128, S)),
                    )
                    # r = 1/sqrt(ssq/D + eps) = exp(-0.5 ln(ssq/D + eps));
                    # ACT-only (DVE InstReciprocal costs ~6.5us per tile)
                    nc.scalar.activation(
                        acc[:], ssq_t[:, off : off + S], AF.Ln,
                        bias=eps_t[:, 0:1], scale=1.0 / D,
                    )
                    nc.scalar.activation(
                        r_bf[:, off : off + S], acc[:], AF.Exp, scale=-0.5
                    )

                def rope(raw, rope_out, cosT, sinT, roff, qoff):
                    for h in range(HL):
                        seg = raw[:, h, :]
                        nc.vector.tensor_tensor(
                            seg, seg, r_bf[:, roff : roff + S], op=OP.mult
                        )
                        if apply_qn:
                            nc.vector.tensor_scalar_mul(
                                seg, seg, qkn_t[:, qoff + h : qoff + h + 1]
                            )
                        # rotate-half = partition shift by 64 -> SBUF-to-SBUF
                        # DMA (DVE lanes cannot cross partitions); the sign of
                        # the rotated half is folded into the sin table.
                        rot = ropep.tile([128, S], BF16, tag="rot")
                        nc.sync.dma_start(rot[0:64, :], raw[64:128, h, :])
                        nc.sync.dma_start(rot[64:128, :], raw[0:64, h, :])
                        tmp1 = ropep.tile([128, S], BF16, tag="rt1")
                        nc.vector.tensor_tensor(tmp1[:], rot[:], sinT[:], op=OP.mult)
                        tmp2 = ropep.tile([128, S], BF16, tag="rt2")
                        nc.vector.tensor_tensor(tmp2[:], seg, cosT[:], op=OP.mult)
                        nc.vector.tensor_tensor(
                            rope_out[:, h, :], tmp2[:], tmp1[:], op=OP.add
                        )

                def v_group(st):
                    psv = psum_p.tile([128, 512], F32, tag="proj")
                    for dt in range(NDT):
                        nc.tensor.matmul(
                            psv[:],
                            xT_t[:, dt, st * 128 : (st + 1) * 128],
                            wv_t[:, dt, :],
                            start=(dt == 0),
                            stop=(dt == NDT - 1),
                        )
                    nc.vector.tensor_copy(out=v_sb[:, st, :], in_=psv[:])

                proj_qk(wq_t, q_raw, qs_t, acc_q, 0, arin_q, arout_q)
                proj_qk(wk_t, k_raw, ks_t, acc_k, S, arin_k, arout_k)
                rope(q_raw, q_rope, cq_t, sq_t, 0, 0)
                for st in range(NST // 2):
                    v_group(st)
                rope(k_raw, k_rope, ck_t, sk_t, S, HL)
                for st in range(NST // 2, NST):
                    v_group(st)

        # ------------------------------------------------------------------
        # Phases 2+3: attention (scores transposed [t, s]) + out projection
        with ExitStack() as p2:
            p2psum = ExitStack()
            scp = p2psum.enter_context(tc.tile_pool(name="sc", bufs=2, space="PSUM"))
            atp = p2psum.enter_context(tc.tile_pool(name="at", bufs=2, space="PSUM"))

            # prefetch the out-projection weights during attention
            wor = woT.rearrange("(t p) o -> p t o", p=128)
            for ct in range(HL):
                nc.sync.dma_start(wo_t[:, ct, :], wor[:, ct, :])

            for h in range(HL):
                at_ps = atp.tile([128, S], F32, tag="at")
                dn_ps = atp.tile([128, S], F32, tag="at")

                def emit_sc(kt, h=h):
                    ps = scp.tile([128, S], F32, tag="sc")
                    lhsT = k_rope[:, h, kt * 128 : (kt + 1) * 128]
                    nc.tensor.matmul(
                        ps[:, 0:512], lhsT, q_rope[:, h, 0:512], start=True, stop=True
                    )
                    nc.tensor.matmul(
                        ps[:, 512:S], lhsT, q_rope[:, h, 512:S], start=True, stop=True
                    )
                    return ps

                prev = emit_sc(0)
                for kt in range(NST):
                    e = ep.tile([128, S], BF16, tag="e")
                    nc.scalar.activation(
                        e[:], prev[:], AF.Exp, bias=mask_t[:, kt : kt + 1], scale=1.0
                    )
                    if kt < NST - 1:
                        prev = emit_sc(kt + 1)
                    first, last = kt == 0, kt == NST - 1
                    nc.tensor.matmul(
                        dn_ps[:, 0:512], ones128[:], e[:, 0:512],
                        start=first, stop=last,
                    )
                    nc.tensor.matmul(
                        dn_ps[:, 512:S], ones128[:], e[:, 512:S],
                        start=first, stop=last,
                    )
                    vt = v_sb[:, kt, h * 128 : (h + 1) * 128]
                    nc.tensor.matmul(
                        at_ps[:, 0:512], vt, e[:, 0:512], start=first, stop=last
                    )
                    nc.tensor.matmul(
                        at_ps[:, 512:S], vt, e[:, 512:S], start=first, stop=last
                    )
                # 1/denom = exp(-ln(denom)); ACT-only, keeps DVE off the
                # per-head critical path
                lt = rcp.tile([128, S], F32, tag="lt")
                nc.scalar.activation(lt[:], dn_ps[:], AF.Ln)
                rc = rcp.tile([128, S], F32, tag="rc")
                nc.scalar.activation(rc[:], lt[:], AF.Exp, scale=-1.0)
                nc.vector.tensor_tensor(attnT[:, h, :], at_ps[:], rc[:], op=OP.mult)

            # out projection (wo_s folded into woT on the host)
            p2psum.close()
            pop = p2.enter_context(tc.tile_pool(name="pop", bufs=4, space="PSUM"))
            for st in range(NST):
                for oc in range(D // 512):
                    ps = pop.tile([128, 512], F32, tag="po")
                    for hh in range(HL):
                        nc.tensor.matmul(
                            ps[:],
                            attnT[:, hh, st * 128 : (st + 1) * 128],
                            wo_t[:, hh, oc * 512 : (oc + 1) * 512],
                            start=(hh == 0),
                            stop=(hh == HL - 1),
                        )
                    og = ostg.tile([128, 512], F32, tag="og")
                    nc.scalar.copy(og[:], ps[:])
                    nc.sync.dma_start(
                        outp[st * 128 : (st + 1) * 128, oc * 512 : (oc + 1) * 512],
                        og[:],
                    )

    _split_excess_waits(nc)
    return nc


_BUILD_CACHE: dict = {}


def kernel(**inputs) -> np.ndarray:
    global LAST_RESULTS
    inp = {k: np.asarray(v) for k, v in inputs.items()}
    hs = inp["hidden_states"].astype(np.float32, copy=False)
    wq, wk, wv, wo = inp["wq"], inp["wk"], inp["wv"], inp["wo"]
    wq_s, wk_s, wv_s, wo_s = (
        inp["wq_s"].astype(np.float32, copy=False),
        inp["wk_s"].astype(np.float32, copy=False),
        inp["wv_s"].astype(np.float32, copy=False),
        inp["wo_s"].astype(np.float32, copy=False),
    )
    qn_w = inp["qn_w"].astype(np.float32, copy=False)
    kn_w = inp["kn_w"].astype(np.float32, copy=False)
    cos = inp["cos"].astype(np.float32, copy=False)
    sin = inp["sin"].astype(np.float32, copy=False)
    lengths = inp["lengths"].astype(np.int64, copy=False)

    apply_qn = not (np.all(qn_w == 1.0) and np.all(kn_w == 1.0))
    if apply_qn not in _BUILD_CACHE:
        _BUILD_CACHE[apply_qn] = build_nc(apply_qn)
    nc = _BUILD_CACHE[apply_qn]

    bf = ml_dtypes.bfloat16
    rsDh = np.float32(1.0 / np.sqrt(Dh))

    # Shared host-side prep.  int8-valued weights are exact in bf16; wv/wo
    # dequant scales are folded into the bf16 weights.
    wqT_f = wq.T.astype(np.float32).astype(bf)
    wkT_f = wk.T.astype(np.float32).astype(bf)
    wvT_f = (wv.T.astype(np.float32).astype(bf).astype(np.float32)
             * wv_s[None, :]).astype(bf)
    woT_f = (wo.T.astype(np.float32).astype(bf).astype(np.float32)
             * wo_s[None, :]).astype(bf)   # [c, o] scaled per output channel o
    xT_b = [np.ascontiguousarray(hs[b].T).astype(bf) for b in range(B)]
    cosT = np.ascontiguousarray(cos.T)           # [Dh, S]
    sinT = np.ascontiguousarray(sin.T)
    sin_sgn = np.concatenate([-sinT[: Dh // 2], sinT[Dh // 2 :]], axis=0)
    cosq_h = np.ascontiguousarray(cosT * rsDh).astype(bf)
    sinq_h = np.ascontiguousarray(sin_sgn * rsDh).astype(bf)
    cosk_h = np.ascontiguousarray(cosT).astype(bf)
    sink_h = np.ascontiguousarray(sin_sgn).astype(bf)
    t_global = np.arange(128)[:, None] + 128 * np.arange(NST)[None, :]
    masks_b = [
        np.where(t_global < int(lengths[b]), 0.0, MASK_NEG).astype(np.float32)
        for b in range(B)
    ]

    in_maps = []
    for c in range(NCORES):
        b, hg = divmod(c, TP)
        cols = slice(DL * hg, DL * hg + DL)
        m = {
            "xT": xT_b[b],
            "wqT": np.ascontiguousarray(wqT_f[:, cols]),
            "wkT": np.ascontiguousarray(wkT_f[:, cols]),
            "wvT": np.ascontiguousarray(wvT_f[:, cols]),
            "woT": np.ascontiguousarray(woT_f[cols, :]),
            "qs": np.ascontiguousarray(wq_s[cols].reshape(NOT_, 128).T),
            "ks": np.ascontiguousarray(wk_s[cols].reshape(NOT_, 128).T),
            "cosq": cosq_h,
            "sinq": sinq_h,
            "cosk": cosk_h,
            "sink": sink_h,
            "maskb": masks_b[b],
        }
        if apply_qn:
            m["qkn"] = np.ascontiguousarray(
                np.concatenate(
                    [qn_w[cols].reshape(HL, 128).T, kn_w[cols].reshape(HL, 128).T],
                    axis=1,
                )
            ).astype(bf)
        in_maps.append(m)

    res = run_bass_kernel_spmd(nc, in_maps, core_ids=list(range(NCORES)))
    LAST_RESULTS = res

    out = np.empty((B, S, D), np.float32)
    for b in range(B):
        np.add(res.results[TP * b]["outp"], res.results[TP * b + 1]["outp"], out=out[b])
        out[b] += res.results[TP * b + 2]["outp"]
        out[b] += res.results[TP * b + 3]["outp"]
    return out


# revision 31
# speedup vs baseline: 1.0066x; 1.0066x over previous
"""Fused int8-dequant attention block on 8 Trainium2 NeuronCores.

Reference computation (per nn_Attention_28183575396503):
    q = rope(rms_norm(x @ wq.T * wq_s, qn_w));  k likewise;  v = x @ wv.T * wv_s
    out = softmax(q k^T / sqrt(Dh) + len_mask) v  @ wo.T * wo_s

Sharding: tensor-parallel over heads (4-way, 4 heads / 512 channels per core)
x data-parallel over batch (2-way).  Core c -> (b = c//4, head-group = c%4).
The RMS-norm sum-of-squares couples all 2048 channels, so each 4-core TP group
does one tiny AllReduce of the per-row partial sums ([1, 2S] fp32).  The
row-parallel out-projection partials are summed on the host (the standard
Megatron all-reduce, done during unshard).

All matmuls run in bf16 (int8 weights are exact in bf16) with fp32 PSUM
accumulation; softmax runs unnormalized-exp (|scores| <~ 7 after RMS norm)
with the length mask folded into the exp bias (per-partition bias on the ACT
op).  The elementwise rope/norm path runs in bf16 for the DVE 2x rate; wv/wo
dequant scales are folded into the bf16 weights on the host.
"""

import numpy as np
import ml_dtypes

import concourse.bass as bass
import concourse.mybir as mybir
import concourse.tile as tile
from concourse.bass_utils import run_bass_kernel_spmd

# ---------------------------------------------------------------------------
# Problem constants (hardcoded per the task contract)
B, S, D, H, Dh = 2, 1024, 2048, 16, 128
EPS = 1e-6
NCORES, TP = 8, 4
DL = D // TP            # 512 local channels per core
HL = H // TP            # 4 local heads per core
NDT = D // 128          # 16 contraction tiles
NOT_ = DL // 128        # 4 local out tiles for q/k
NST = S // 128          # 8 sequence tiles
NSC = S // 512          # 2 sequence chunks of 512
BF16 = mybir.dt.bfloat16
F32 = mybir.dt.float32
MASK_NEG = -30000.0

LAST_RESULTS = None     # BassKernelResults of the most recent kernel() call


# ---------------------------------------------------------------------------
# Workaround: this walrus build accepts only ONE sync-wait per instruction,
# but Tile freely emits several (and puts every engine's tail wait on the
# kernel-end Drain).  Post-pass: spill excess waits onto same-engine NoOps
# inserted immediately before the instruction (engines run in program order,
# so waiting on the NoOps first is equivalent).
_MAX_WAITS = 1


def _split_excess_waits(nc: bass.Bass) -> None:
    ctr = 0
    for f in nc.m.functions:
        for bb in f.blocks:
            il = bb.instructions
            new = []
            for inst in il:
                si = inst.sync_info
                ow = list(si.on_wait) if si is not None and si.on_wait else []
                if len(ow) > _MAX_WAITS:
                    for w in ow[_MAX_WAITS:]:
                        ctr += 1
                        new.append(
                            mybir.InstNoOp(
                                name=f"{inst.name}_sw{ctr}",
                                sync_info=mybir.SyncInfo(on_wait=[w], on_update=[]),
                                bass_nofuse=True,
                                engine=inst.engine,
                            )
                        )
                    inst.sync_info = mybir.SyncInfo(
                        on_wait=ow[:_MAX_WAITS],
                        on_update=list(si.on_update) if si.on_update else [],
                    )
                new.append(inst)
            il[:] = new


# ---------------------------------------------------------------------------
def build_nc(apply_qn: bool) -> bass.Bass:
    from contextlib import ExitStack

    AF = mybir.ActivationFunctionType
    OP = mybir.AluOpType

    nc = bass.Bass()
    xT = nc.dram_tensor("xT", [D, S], BF16, kind="ExternalInput")
    wqT = nc.dram_tensor("wqT", [D, DL], BF16, kind="ExternalInput")
    wkT = nc.dram_tensor("wkT", [D, DL], BF16, kind="ExternalInput")
    wvT = nc.dram_tensor("wvT", [D, DL], BF16, kind="ExternalInput")
    woT = nc.dram_tensor("woT", [DL, D], BF16, kind="ExternalInput")
    qs = nc.dram_tensor("qs", [128, NOT_], F32, kind="ExternalInput")
    ks = nc.dram_tensor("ks", [128, NOT_], F32, kind="ExternalInput")
    cosq = nc.dram_tensor("cosq", [128, S], BF16, kind="ExternalInput")
    sinq = nc.dram_tensor("sinq", [128, S], BF16, kind="ExternalInput")
    cosk = nc.dram_tensor("cosk", [128, S], BF16, kind="ExternalInput")
    sink = nc.dram_tensor("sink", [128, S], BF16, kind="ExternalInput")
    maskb = nc.dram_tensor("maskb", [128, NST], F32, kind="ExternalInput")
    if apply_qn:
        qkn = nc.dram_tensor("qkn", [128, 2 * HL], BF16, kind="ExternalInput")
    outp = nc.dram_tensor("outp", [S, D], F32, kind="ExternalOutput")

    with tile.TileContext(nc) as tc, ExitStack() as ctx:
        cpool = ctx.enter_context(tc.tile_pool(name="const", bufs=1))
        dram = ctx.enter_context(tc.tile_pool(name="dram", bufs=1, space="DRAM"))

        # Long-lived SBUF tensors
        q_rope = cpool.tile([128, HL, S], BF16)      # roped/normed q^T per head
        k_rope = cpool.tile([128, HL, S], BF16)
        v_sb = cpool.tile([128, NST, DL], BF16)      # v in [s, o] layout
        attnT = cpool.tile([128, HL, S], BF16)       # attn^T per head
        qs_t = cpool.tile([128, NOT_], F32)
        ks_t = cpool.tile([128, NOT_], F32)
        mask_t = cpool.tile([128, NST], F32)
        ones1 = cpool.tile([128, 1], F32)
        ones128 = cpool.tile([128, 128], F32)
        eps_t = cpool.tile([128, 1], F32)
        if apply_qn:
            qkn_t = cpool.tile([128, 2 * HL], BF16)

        # Phase-2/3 SBUF pools hoisted above phase 1 so their tiles never
        # overlap phase-1 addresses (stack allocator WAR deps would chain
        # attention starts behind the whole rope phase otherwise).
        ep = ctx.enter_context(tc.tile_pool(name="expp", bufs=3))
        rcp = ctx.enter_context(tc.tile_pool(name="recipp", bufs=1))
        wop = ctx.enter_context(tc.tile_pool(name="wop", bufs=1))
        ostg = ctx.enter_context(tc.tile_pool(name="ostg", bufs=3))
        sep = ctx.enter_context(tc.tile_pool(name="sumexp", bufs=2))
        wo_t = wop.tile([128, HL, D], BF16)

        nc.vector.memset(ones1[:], 1.0)
        nc.vector.memset(ones128[:], 1.0)
        nc.vector.memset(eps_t[:], EPS)
        nc.sync.dma_start(qs_t[:], qs[:, :])
        nc.sync.dma_start(ks_t[:], ks[:, :])
        nc.sync.dma_start(mask_t[:], maskb[:, :])
        if apply_qn:
            nc.sync.dma_start(qkn_t[:], qkn[:, :])

        arin_q = dram.tile([1, S], F32)
        arout_q = dram.tile([1, S], F32)
        arin_k = dram.tile([1, S], F32)
        arout_k = dram.tile([1, S], F32)

        # ------------------------------------------------------------------
        # Phase 1: QKV projections, distributed RMS-norm stats, RoPE
        with ExitStack() as p1:
            opool = p1.enter_context(tc.tile_pool(name="p1o", bufs=1))
            q_raw = opool.tile([128, NOT_, S], BF16)
            k_raw = opool.tile([128, NOT_, S], BF16)
            ssq_t = opool.tile([128, 2 * S], F32)
            acc_q = opool.tile([128, S], F32)
            acc_k = opool.tile([128, S], F32)

            # ---- p1a: projections + ssq AllReduces + RoPE, FIFO-balanced:
            # DVE: q-epi, k-epi, rope-q, v-copies 0-3, rope-k, v-copies 4-7
            # ACT: ln/exp (r chains) only
            # SP DMA ring: input loads, cos/sin, rot shifts (never blocked)
            # GpSimd: AR staging/trigger/broadcast (the AR-dependent chain)
            with ExitStack() as p1a:
                apool = p1a.enter_context(tc.tile_pool(name="p1a", bufs=1))
                xT_t = apool.tile([128, NDT, S], BF16)
                wq_t = apool.tile([128, NDT, DL], BF16)
                wk_t = apool.tile([128, NDT, DL], BF16)
                wv_t = apool.tile([128, NDT, DL], BF16)
                cq_t = apool.tile([128, S], BF16)
                sq_t = apool.tile([128, S], BF16)
                ck_t = apool.tile([128, S], BF16)
                sk_t = apool.tile([128, S], BF16)

                xTr = xT.rearrange("(t p) s -> p t s", p=128)
                wqr = wqT.rearrange("(t p) o -> p t o", p=128)
                wkr = wkT.rearrange("(t p) o -> p t o", p=128)
                wvr = wvT.rearrange("(t p) o -> p t o", p=128)
                for dt in range(NDT):
                    nc.sync.dma_start(xT_t[:, dt, :], xTr[:, dt, :])
                    nc.sync.dma_start(wq_t[:, dt, :], wqr[:, dt, :])
                    nc.sync.dma_start(wk_t[:, dt, :], wkr[:, dt, :])
                    nc.sync.dma_start(wv_t[:, dt, :], wvr[:, dt, :])
                nc.sync.dma_start(cq_t[:], cosq[:, :])
                nc.sync.dma_start(sq_t[:], sinq[:, :])
                nc.sync.dma_start(ck_t[:], cosk[:, :])
                nc.sync.dma_start(sk_t[:], sink[:, :])

                psum_p = p1a.enter_context(
                    tc.tile_pool(name="psA", bufs=4, space="PSUM")
                )
                psum_s = p1a.enter_context(
                    tc.tile_pool(name="psS", bufs=1, space="PSUM")
                )
                sqpool = p1a.enter_context(
                    tc.tile_pool(name="sqp", bufs=2, space="PSUM")
                )
                ropep = p1a.enter_context(tc.tile_pool(name="rope", bufs=2))

                def proj_qk(wt, raw, sc_t, acc, off, ari, aro):
                    for ot in range(NOT_):
                        for sc_ in range(NSC):
                            ps = psum_p.tile([128, 512], F32, tag="proj")
                            for dt in range(NDT):
                                nc.tensor.matmul(
                                    ps[:],
                                    wt[:, dt, ot * 128 : (ot + 1) * 128],
                                    xT_t[:, dt, sc_ * 512 : (sc_ + 1) * 512],
                                    start=(dt == 0),
                                    stop=(dt == NDT - 1),
                                )
                            seg = raw[:, ot, sc_ * 512 : (sc_ + 1) * 512]
                            nc.vector.tensor_scalar_mul(
                                seg, ps[:], sc_t[:, ot : ot + 1]
                            )
                            accseg = acc[:, sc_ * 512 : (sc_ + 1) * 512]
                            if ot == 0:
                                nc.vector.tensor_tensor(
                                    accseg, seg, seg, op=OP.mult
                                )
                            else:
                                sq = sqpool.tile([128, 512], F32, tag="sq")
                                nc.vector.tensor_tensor(sq[:], seg, seg, op=OP.mult)
                                nc.vector.tensor_tensor(
                                    accseg, accseg, sq[:], op=OP.add
                                )
                    # partial sum of squares over the 512 local channels
                    pssq = psum_s.tile([1, S], F32, tag="ssq")
                    for sc_ in range(NSC):
                        nc.tensor.matmul(
                            pssq[0:1, sc_ * 512 : (sc_ + 1) * 512],
                            ones1[:],
                            acc[:, sc_ * 512 : (sc_ + 1) * 512],
                            start=True,
                            stop=True,
                        )
                    # acc row 0 is dead now; reuse it to stage psum -> DRAM.
                    # The whole AR-dependent chain goes through GpSimd queues
                    # so it can never head-of-line-block the SP DMA ring.
                    nc.vector.tensor_copy(out=acc[0:1, :], in_=pssq[0:1, :])
                    nc.gpsimd.dma_start(ari[0:1, :], acc[0:1, :])
                    nc.gpsimd.collective_compute(
                        "AllReduce",
                        OP.add,
                        replica_groups=[[0, 1, 2, 3], [4, 5, 6, 7]],
                        ins=[ari.opt()],
                        outs=[aro.opt()],
                    )
                    nc.gpsimd.dma_start(
                        ssq_t[:, off : off + S],
                        aro[0:1, :].broadcast_to((128, S)),
                    )
                    # r = 1/sqrt(ssq/D + eps) = exp(-0.5 ln(ssq/D + eps));
                    # ACT-only (DVE InstReciprocal costs ~6.5us per tile)
                    nc.scalar.activation(
                        acc[:], ssq_t[:, off : off + S], AF.Ln,
                        bias=eps_t[:, 0:1], scale=1.0 / D,
                    )
                    nc.scalar.activation(
                        ssq_t[:, off : off + S], acc[:], AF.Exp, scale=-0.5
                    )

                def rope(raw, rope_out, cosT, sinT, roff, qoff):
                    # Fold r into the cos/sin tables (r is constant across
                    # partitions), so the rotate-half DMAs can run on the
                    # raw (pre-norm) values as soon as the projection lands.
                    nc.vector.tensor_tensor(
                        cosT[:], cosT[:], ssq_t[:, roff : roff + S], op=OP.mult
                    )
                    nc.vector.tensor_tensor(
                        sinT[:], sinT[:], ssq_t[:, roff : roff + S], op=OP.mult
                    )
                    for h in range(HL):
                        seg = raw[:, h, :]
                        if apply_qn:
                            nc.vector.tensor_scalar_mul(
                                seg, seg, qkn_t[:, qoff + h : qoff + h + 1]
                            )
                        # rotate-half = partition shift by 64 -> SBUF-to-SBUF
                        # DMA (DVE lanes cannot cross partitions); the sign of
                        # the rotated half is folded into the sin table.
                        rot = ropep.tile([128, S], BF16, tag="rot")
                        nc.sync.dma_start(rot[0:64, :], raw[64:128, h, :])
                        nc.sync.dma_start(rot[64:128, :], raw[0:64, h, :])
                        tmp1 = ropep.tile([128, S], BF16, tag="rt1")
                        nc.vector.tensor_tensor(tmp1[:], rot[:], sinT[:], op=OP.mult)
                        nc.vector.tensor_tensor(rot[:], seg, cosT[:], op=OP.mult)
                        nc.vector.tensor_tensor(
                            rope_out[:, h, :], rot[:], tmp1[:], op=OP.add
                        )

                def v_group(st):
                    psv = psum_p.tile([128, 512], F32, tag="proj")
                    for dt in range(NDT):
                        nc.tensor.matmul(
                            psv[:],
                            xT_t[:, dt, st * 128 : (st + 1) * 128],
                            wv_t[:, dt, :],
                            start=(dt == 0),
                            stop=(dt == NDT - 1),
                        )
                    nc.vector.tensor_copy(out=v_sb[:, st, :], in_=psv[:])

                proj_qk(wq_t, q_raw, qs_t, acc_q, 0, arin_q, arout_q)
                proj_qk(wk_t, k_raw, ks_t, acc_k, S, arin_k, arout_k)
                rope(q_raw, q_rope, cq_t, sq_t, 0, 0)
                for st in range(NST // 2):
                    v_group(st)
                rope(k_raw, k_rope, ck_t, sk_t, S, HL)
                for st in range(NST // 2, NST):
                    v_group(st)

        # ------------------------------------------------------------------
        # Phases 2+3: attention (scores transposed [t, s]) + out projection
        with ExitStack() as p2:
            p2psum = ExitStack()
            scp = p2psum.enter_context(tc.tile_pool(name="sc", bufs=2, space="PSUM"))
            atp = p2psum.enter_context(tc.tile_pool(name="at", bufs=2, space="PSUM"))

            # prefetch the out-projection weights during attention
            wor = woT.rearrange("(t p) o -> p t o", p=128)
            for ct in range(HL):
                nc.sync.dma_start(wo_t[:, ct, :], wor[:, ct, :])

            for h in range(HL):
                at_ps = atp.tile([128, S], F32, tag="at")
                acc = sep.tile([128, S], F32, tag="se")

                def emit_sc(kt, h=h):
                    ps = scp.tile([128, S], F32, tag="sc")
                    lhsT = k_rope[:, h, kt * 128 : (kt + 1) * 128]
                    nc.tensor.matmul(
                        ps[:, 0:512], lhsT, q_rope[:, h, 0:512], start=True, stop=True
                    )
                    nc.tensor.matmul(
                        ps[:, 512:S], lhsT, q_rope[:, h, 512:S], start=True, stop=True
                    )
                    return ps

                prev = emit_sc(0)
                for kt in range(NST):
                    e = ep.tile([128, S], BF16, tag="e")
                    nc.scalar.activation(
                        e[:], prev[:], AF.Exp, bias=mask_t[:, kt : kt + 1], scale=1.0
                    )
                    if kt < NST - 1:
                        prev = emit_sc(kt + 1)
                    first, last = kt == 0, kt == NST - 1
                    # denominator partials accumulate on DVE (the attention
                    # phase is PE-limited; DVE is idle here)
                    if first:
                        nc.vector.tensor_copy(out=acc[:], in_=e[:])
                    else:
                        nc.vector.tensor_tensor(acc[:], acc[:], e[:], op=OP.add)
                    vt = v_sb[:, kt, h * 128 : (h + 1) * 128]
                    nc.tensor.matmul(
                        at_ps[:, 0:512], vt, e[:, 0:512], start=first, stop=last
                    )
                    nc.tensor.matmul(
                        at_ps[:, 512:S], vt, e[:, 512:S], start=first, stop=last
                    )
                dn_ps = scp.tile([128, S], F32, tag="sc")
                nc.tensor.matmul(
                    dn_ps[:, 0:512], ones128[:], acc[:, 0:512], start=True, stop=True
                )
                nc.tensor.matmul(
                    dn_ps[:, 512:S], ones128[:], acc[:, 512:S], start=True, stop=True
                )
                # 1/denom = exp(-ln(denom)); ACT-only, keeps DVE off the
                # per-head critical path
                lt = rcp.tile([128, S], F32, tag="lt")
                nc.scalar.activation(lt[:], dn_ps[:], AF.Ln)
                rc = rcp.tile([128, S], F32, tag="rc")
                nc.scalar.activation(rc[:], lt[:], AF.Exp, scale=-1.0)
                nc.vector.tensor_tensor(attnT[:, h, :], at_ps[:], rc[:], op=OP.mult)

            # out projection (wo_s folded into woT on the host)
            p2psum.close()
            pop = p2.enter_context(tc.tile_pool(name="pop", bufs=4, space="PSUM"))
            for st in range(NST):
                for oc in range(D // 512):
                    ps = pop.tile([128, 512], F32, tag="po")
                    for hh in range(HL):
                        nc.tensor.matmul(
                            ps[:],
                            attnT[:, hh, st * 128 : (st + 1) * 128],
                            wo_t[:, hh, oc * 512 : (oc + 1) * 512],
                            start=(hh == 0),
                            stop=(hh == HL - 1),
                        )
                    og = ostg.tile([128, 512], F32, tag="og")
                    nc.scalar.copy(og[:], ps[:])
                    nc.sync.dma_start(
                        outp[st * 128 : (st + 1) * 128, oc * 512 : (oc + 1) * 512],
                        og[:],
                    )

    _split_excess_waits(nc)
    return nc


_BUILD_CACHE: dict = {}


def kernel(**inputs) -> np.ndarray:
    global LAST_RESULTS
    inp = {k: np.asarray(v) for k, v in inputs.items()}
    hs = inp["hidden_states"].astype(np.float32, copy=False)
    wq, wk, wv, wo = inp["wq"], inp["wk"], inp["wv"], inp["wo"]
    wq_s, wk_s, wv_s, wo_s = (
        inp["wq_s"].astype(np.float32, copy=False),
        inp["wk_s"].astype(np.float32, copy=False),
        inp["wv_s"].astype(np.float32, copy=False),
        inp["wo_s"].astype(np.float32, copy=False),
    )
    qn_w = inp["qn_w"].astype(np.float32, copy=False)
    kn_w = inp["kn_w"].astype(np.float32, copy=False)
    cos = inp["cos"].astype(np.float32, copy=False)
    sin = inp["sin"].astype(np.float32, copy=False)
    lengths = inp["lengths"].astype(np.int64, copy=False)

    apply_qn = not (np.all(qn_w == 1.0) and np.all(kn_w == 1.0))
    if apply_qn not in _BUILD_CACHE:
        _BUILD_CACHE[apply_qn] = build_nc(apply_qn)
    nc = _BUILD_CACHE[apply_qn]

    bf = ml_dtypes.bfloat16
    rsDh = np.float32(1.0 / np.sqrt(Dh))

    # Shared host-side prep.  int8-valued weights are exact in bf16; wv/wo
    # dequant scales are folded into the bf16 weights.
    wqT_f = wq.T.astype(np.float32).astype(bf)
    wkT_f = wk.T.astype(np.float32).astype(bf)
    wvT_f = (wv.T.astype(np.float32).astype(bf).astype(np.float32)
             * wv_s[None, :]).astype(bf)
    woT_f = (wo.T.astype(np.float32).astype(bf).astype(np.float32)
             * wo_s[None, :]).astype(bf)   # [c, o] scaled per output channel o
    xT_b = [np.ascontiguousarray(hs[b].T).astype(bf) for b in range(B)]
    cosT = np.ascontiguousarray(cos.T)           # [Dh, S]
    sinT = np.ascontiguousarray(sin.T)
    sin_sgn = np.concatenate([-sinT[: Dh // 2], sinT[Dh // 2 :]], axis=0)
    cosq_h = np.ascontiguousarray(cosT * rsDh).astype(bf)
    sinq_h = np.ascontiguousarray(sin_sgn * rsDh).astype(bf)
    cosk_h = np.ascontiguousarray(cosT).astype(bf)
    sink_h = np.ascontiguousarray(sin_sgn).astype(bf)
    t_global = np.arange(128)[:, None] + 128 * np.arange(NST)[None, :]
    masks_b = [
        np.where(t_global < int(lengths[b]), 0.0, MASK_NEG).astype(np.float32)
        for b in range(B)
    ]

    in_maps = []
    for c in range(NCORES):
        b, hg = divmod(c, TP)
        cols = slice(DL * hg, DL * hg + DL)
        m = {
            "xT": xT_b[b],
            "wqT": np.ascontiguousarray(wqT_f[:, cols]),
            "wkT": np.ascontiguousarray(wkT_f[:, cols]),
            "wvT": np.ascontiguousarray(wvT_f[:, cols]),
            "woT": np.ascontiguousarray(woT_f[cols, :]),
            "qs": np.ascontiguousarray(wq_s[cols].reshape(NOT_, 128).T),
            "ks": np.ascontiguousarray(wk_s[cols].reshape(NOT_, 128).T),
            "cosq": cosq_h,
            "sinq": sinq_h,
            "cosk": cosk_h,
            "sink": sink_h,
            "maskb": masks_b[b],
        }
        if apply_qn:
            m["qkn"] = np.ascontiguousarray(
                np.concatenate(
                    [qn_w[cols].reshape(HL, 128).T, kn_w[cols].reshape(HL, 128).T],
                    axis=1,
                )
            ).astype(bf)
        in_maps.append(m)

    res = run_bass_kernel_spmd(nc, in_maps, core_ids=list(range(NCORES)))
    LAST_RESULTS = res

    out = np.empty((B, S, D), np.float32)
    for b in range(B):
        np.add(res.results[TP * b]["outp"], res.results[TP * b + 1]["outp"], out=out[b])
        out[b] += res.results[TP * b + 2]["outp"]
        out[b] += res.results[TP * b + 3]["outp"]
    return out
